# revision 11
# baseline (speedup 1.0000x reference)
"""NetVLAD Trainium2 Bass kernel, SPMD over 8 NeuronCores.

Contract: kernel(x, Wc, C) takes the FULL inputs
  x  [16, 56, 56, 512] f32, Wc [512, 32] f32, C [512, 32] f32
and returns the FULL output [16, 16384] f32 (matches reference()).

Sharding: data-parallel over batch — 2 samples per core; Wc/C replicated.

v5 design:
  - x is uploaded TWICE in reduced precision: pixel-major xb in FP8-e4m3
    (moving operand of mm2 acc += a^T x; fp8 error averages out over the
    3136-pixel reduction) and host-pre-transposed xt in BF16 with
    xt[t, p, j, q] = x[128t+q, 128j+p] (stationary operand of mm1
    s = x Wc; the softmax needs bf16 logits). TRN2 matmul accepts mixed
    bf16 stationary x fp8 moving (probed bit-exact vs float emulation).
    ~10 MB/core of DMA (~28 us at 358 GB/s). No PE transposes or PSUM
    copies in the main loop.
  - xb flows through the Sync DMA queue, xt through the GpSimd queue, in
    superbatches of 4-8 tiles so the ~0.7 us per-DMA completion latency
    amortizes while the first compute starts early.
  - each sample is PADDED from 3136 to 3200 pixels (25 tiles of 128) with
    x=0. Pad pixels contribute 0 to a^T x and exactly 1/32 per cluster to
    the softmax weights, so a_sum overcounts by the constant 64/32 = 2.0,
    subtracted for free via the bias of the a_sum PSUM->SBUF copy. Every
    4-tile group then lies inside one sample: no row-split matmuls.
  - tiles are processed in GROUPS of 4: the four s tiles accumulate into
    one PSUM bank [128, 4, 32] and the softmax runs once per group
    (1 ACT exp + 1 DVE reduce + 1 DVE recip + 1 broadcast mul),
    amortizing per-instruction overheads 4x. a_sum is one matmul per
    group (lhsT = a4 [128, 4*32], rhs = ones) onto 128 PSUM partitions;
    a tiny fold matmul (eye(32) tiled 4x) in the epilogue sums the 4
    tile-slots.
  - mm2 emission lags mm1 by ~2 groups and is interleaved one unit per
    mm1-tile so the 215 ns acc-matmul streams hide mm1's LDWEIGHTS and
    the PE never waits on the softmax chain. Softmax skips
    max-subtraction (|s| <= ~10 is f32-exp-safe).
  - epilogue per sample: vT = C^T*a_sum + acc fused in one
    scalar_tensor_tensor, PE-transpose to [d, k], ACT square + DVE reduce
    for the intra-norm; the global L2 norm of the intra-normalized matrix
    is exactly sqrt(512), folded analytically into the Sqrt scale.
Measured end-to-end relative error vs the f32 reference ~1.0e-2
(fp8 on the a^T x path; the all-bf16 variant measures 1.8e-3).
"""
import sys

if '/opt/trn_rl_repo' not in sys.path:
    sys.path.insert(0, '/opt/trn_rl_repo')

from contextlib import ExitStack

import numpy as np

N_PIX = 3136
N_SAMP = 2
P = 128
NTS = 25              # tiles per sample (padded to 3200 px)
NT = NTS * N_SAMP     # 50
N_ROWSP = NT * P      # 6400 padded rows
D = 512
K = 32
DC = D // P           # 4
N_CORES = 8
MAXG = 4
MAXB = 8
KEEP = 10             # pending mm2 closures kept in flight (~2 groups)

# DMA superbatches (sample, first local tile, n tiles); groups of 4 tiles
# (last group of each sample is the single padded tile 24) subdivide them.
BATCHES = []
for _s in range(N_SAMP):
    BATCHES += [(_s, 0, 4), (_s, 4, 8), (_s, 12, 8), (_s, 20, 5)]

_cache = {}


def _build():
    import concourse.bacc as bacc
    import concourse.mybir as mybir
    import concourse.tile as tile
    from concourse.bass import ts

    F32 = mybir.dt.float32
    BF16 = mybir.dt.bfloat16
    FP8 = mybir.dt.float8e4
    MULT = mybir.AluOpType.mult
    ADD = mybir.AluOpType.add

    nc = bacc.Bacc("TRN2", target_bir_lowering=False, debug=False)

    xb = nc.declare_dram_parameter("xb", [N_ROWSP, D], FP8, isOutput=False)
    xt = nc.declare_dram_parameter("xt", [NT, P, DC, P], BF16,
                                   isOutput=False)
    wc = nc.declare_dram_parameter("wc", [D, K], BF16, isOutput=False)
    ct = nc.declare_dram_parameter("ct", [K, D], F32, isOutput=False)
    id32 = nc.declare_dram_parameter("id32", [K, K], F32, isOutput=False)
    ones2 = nc.declare_dram_parameter("ones2", [P, 2], BF16, isOutput=False)
    wfold = nc.declare_dram_parameter("wfold", [P, K], F32, isOutput=False)
    out = nc.declare_dram_parameter("out", [N_SAMP, DC, P, K], F32,
                                    isOutput=True)
    xb, xt, wc, ct, id32, ones2, wfold, out = (
        xb.ap(), xt.ap(), wc.ap(), ct.ap(), id32.ap(), ones2.ap(),
        wfold.ap(), out.ap())
    xb_r = xb.rearrange("(t p) d -> p t d", p=P)      # [P, NT, D]
    xt_r = xt.rearrange("t p j q -> p t j q")         # [P, NT, DC, P]

    with tile.TileContext(nc) as tc, ExitStack() as ctx:
        consts = ctx.enter_context(tc.tile_pool(name="consts", bufs=1))
        xbpool = ctx.enter_context(tc.tile_pool(name="xbpool", bufs=3))
        xtpool = ctx.enter_context(tc.tile_pool(name="xtpool", bufs=3))
        small = ctx.enter_context(tc.tile_pool(name="small", bufs=6))
        epil = ctx.enter_context(tc.tile_pool(name="epil", bufs=2))
        ps_s = ctx.enter_context(tc.tile_pool(name="ps_s", bufs=3,
                                              space="PSUM"))
        ps_acc = ctx.enter_context(tc.tile_pool(name="ps_acc", bufs=2,
                                                space="PSUM"))
        ps_asum = ctx.enter_context(tc.tile_pool(name="ps_asum", bufs=2,
                                                 space="PSUM"))

        wc_sb = consts.tile([P, DC, K], BF16)
        nc.sync.dma_start(out=wc_sb, in_=wc.rearrange("(c p) k -> p c k", p=P))
        ct_sb = consts.tile([K, D], F32)
        nc.sync.dma_start(out=ct_sb, in_=ct)
        id32_sb = consts.tile([K, K], F32)
        nc.sync.dma_start(out=id32_sb, in_=id32)
        ones_sb = consts.tile([P, 2], BF16)
        nc.sync.dma_start(out=ones_sb, in_=ones2)
        wfold_sb = consts.tile([P, K], F32)
        nc.sync.dma_start(out=wfold_sb, in_=wfold)

        acc = [ps_acc.tile([K, D], F32, name=f"acc{s}", tag="acc")
               for s in range(N_SAMP)]
        asum_ps = [ps_asum.tile([P, 2], F32, name=f"asumps{s}", tag="asum_ps")
                   for s in range(N_SAMP)]

        def epilogue(s):
            # fold the 4 tile-slot blocks of a_sum and subtract the pad
            # contribution (64 pad px * 1/32 = 2.0 total, 0.5 per block)
            asum_sb = epil.tile([P, 2], F32, name=f"asb{s}", tag="asb")
            nc.scalar.activation(asum_sb, asum_ps[s],
                                 mybir.ActivationFunctionType.Copy,
                                 bias=-0.5)
            asum_f = ps_s.tile([K, 2], F32, name=f"af{s}", tag="sps")
            nc.tensor.matmul(asum_f, wfold_sb, asum_sb,
                             start=True, stop=True, skip_group_check=True)
            # vT = C^T * a_sum + acc, fused on DVE
            vt_sb = epil.tile([K, D], F32, name=f"vt{s}", tag="vt")
            nc.vector.scalar_tensor_tensor(vt_sb, ct_sb, asum_f[:, 0:1],
                                           acc[s][:, :], op0=MULT, op1=ADD)
            v_ps = ps_s.tile([P, DC, K], F32, name=f"vps{s}", tag="sps")
            for j in range(DC):
                nc.tensor.transpose(v_ps[:, j, :], vt_sb[:, ts(j, P)], id32_sb)
            vsq = epil.tile([P, DC, K], F32, name=f"vsq{s}", tag="vsq")
            nc.scalar.activation(vsq, v_ps,
                                 mybir.ActivationFunctionType.Square)
            ssq = epil.tile([P, DC], F32, name=f"ssq{s}", tag="ssq")
            nc.vector.reduce_sum(ssq, vsq, axis=mybir.AxisListType.X)
            snorm = epil.tile([P, DC], F32, name=f"sn{s}", tag="sn")
            nc.scalar.activation(snorm, ssq,
                                 mybir.ActivationFunctionType.Sqrt,
                                 scale=float(D))
            rmult = epil.tile([P, DC], F32, name=f"rm{s}", tag="rm")
            nc.vector.reciprocal(rmult, snorm)
            v_sb = epil.tile([P, DC, K], F32, name=f"v{s}", tag="v")
            nc.vector.tensor_mul(v_sb, v_ps, rmult.to_broadcast([P, DC, K]))
            nc.sync.dma_start(out=out[s].rearrange("c p k -> p c k"),
                              in_=v_sb)

        pending = []

        def pop_pending(keep):
            while len(pending) > keep:
                pending.pop(0)()

        def make_acc(s, tl, a4, u, xb_t, bu):
            def emit():
                nc.tensor.matmul(acc[s][:, :], a4[:, u, :], xb_t[:, bu, :],
                                 start=(tl == 0), stop=(tl == NTS - 1),
                                 skip_group_check=True)
            return emit

        def make_asum(s, tl0, sz, a4):
            def emit():
                nc.tensor.matmul(asum_ps[s][0:sz * K, :], a4[:, 0:sz, :],
                                 ones_sb,
                                 start=(tl0 == 0), stop=(tl0 + sz == NTS),
                                 skip_group_check=True)
                if tl0 + sz == NTS:
                    epilogue(s)
            return emit

        for s, btl0, bsz in BATCHES:
            t0 = s * NTS + btl0
            xb_t = xbpool.tile([P, MAXB, D], FP8, name="xb_t")
            nc.sync.dma_start(out=xb_t[:, 0:bsz, :],
                              in_=xb_r[:, t0:t0 + bsz, :])
            xt_t = xtpool.tile([P, MAXB, DC, P], BF16, name="xt_t")
            nc.gpsimd.dma_start(out=xt_t[:, 0:bsz, :, :],
                                in_=xt_r[:, t0:t0 + bsz, :, :])
            for g0 in range(0, bsz, MAXG):
                sz = min(MAXG, bsz - g0)
                tl0 = btl0 + g0
                s_ps = ps_s.tile([P, MAXG, K], F32, name="s_ps", tag="sps")
                for u in range(sz):
                    for j in range(DC):
                        nc.tensor.matmul(s_ps[:, u, :],
                                         xt_t[:, g0 + u, j, :],
                                         wc_sb[:, j, :],
                                         start=(j == 0), stop=(j == DC - 1),
                                         skip_group_check=True)
                    pop_pending(KEEP)
                exp4 = small.tile([P, MAXG, K], F32, name="exp4")
                nc.scalar.activation(exp4[:, 0:sz, :], s_ps[:, 0:sz, :],
                                     mybir.ActivationFunctionType.Exp)
                sum4 = small.tile([P, MAXG], F32, name="sum4")
                nc.vector.reduce_sum(sum4[:, 0:sz], exp4[:, 0:sz, :],
                                     axis=mybir.AxisListType.X)
                rcp4 = small.tile([P, MAXG], F32, name="rcp4")
                nc.vector.reciprocal(rcp4[:, 0:sz], sum4[:, 0:sz])
                a4 = small.tile([P, MAXG, K], BF16, name="a4")
                nc.vector.tensor_mul(
                    a4[:, 0:sz, :], exp4[:, 0:sz, :],
                    rcp4[:, 0:sz].to_broadcast([P, sz, K]))
                for u in range(sz):
                    pending.append(
                        make_acc(s, tl0 + u, a4, u, xb_t, g0 + u))
                pending.append(make_asum(s, tl0, sz, a4))
        pop_pending(0)

    nc.finalize()
    return nc


def _get_nc():
    if "nc" not in _cache:
        _cache["nc"] = _build()
    return _cache["nc"]


def make_maps(x, Wc, C):
    """Host-side prep: shard over batch, pad samples to 3200 px, build
    fp8 xb / pre-transposed bf16 xt."""
    import ml_dtypes

    bf16 = ml_dtypes.bfloat16
    fp8 = ml_dtypes.float8_e4m3
    x = np.asarray(x, dtype=np.float32)
    wc_h = np.asarray(Wc, dtype=np.float32).astype(bf16)
    ct_h = np.ascontiguousarray(np.asarray(C, dtype=np.float32).T)
    id32 = np.eye(K, dtype=np.float32)
    ones2 = np.ones((P, 2), dtype=bf16)
    wfold_h = np.tile(np.eye(K, dtype=np.float32), (DC, 1))

    B = x.shape[0]
    per = B // N_CORES
    maps = []
    for i in range(N_CORES):
        xs = x[i * per:(i + 1) * per].reshape(per, N_PIX, D)
        xp = np.zeros((per, NTS * P, D), dtype=np.float32)
        xp[:, :N_PIX, :] = xs
        xp = xp.reshape(N_ROWSP, D)
        # xt[t, p, j, q] = xp[128t+q, 128j+p]
        xtt = np.ascontiguousarray(
            xp.reshape(NT, P, DC, P).transpose(0, 3, 2, 1).astype(bf16))
        maps.append({"xb": np.ascontiguousarray(xp.astype(fp8)), "xt": xtt,
                     "wc": wc_h, "ct": ct_h, "id32": id32, "ones2": ones2,
                     "wfold": wfold_h})
    return maps


def kernel(x, Wc, C):
    from concourse.bass_utils import run_bass_kernel_spmd

    nc = _get_nc()
    maps = make_maps(x, Wc, C)
    res = run_bass_kernel_spmd(nc, maps, list(range(N_CORES)))
    outs = [r["out"].reshape(N_SAMP, D * K) for r in res.results]
    return np.concatenate(outs, axis=0)


# revision 14
# speedup vs baseline: 1.1570x; 1.1570x over previous
"""NetVLAD Trainium2 Bass kernel, SPMD over 8 NeuronCores.

Contract: kernel(x, Wc, C) takes the FULL inputs
  x  [16, 56, 56, 512] f32, Wc [512, 32] f32, C [512, 32] f32
and returns the FULL output [16, 16384] f32 (matches reference()).

Sharding: data-parallel over batch — 2 samples per core; Wc/C replicated.

v5 design:
  - x is uploaded TWICE in reduced precision: pixel-major xb in FP8-e4m3
    (moving operand of mm2 acc += a^T x; fp8 error averages out over the
    3136-pixel reduction) and host-pre-transposed xt in BF16 with
    xt[t, p, j, q] = x[128t+q, 128j+p] (stationary operand of mm1
    s = x Wc; the softmax needs bf16 logits). TRN2 matmul accepts mixed
    bf16 stationary x fp8 moving (probed bit-exact vs float emulation).
    ~10 MB/core of DMA (~28 us at 358 GB/s). No PE transposes or PSUM
    copies in the main loop.
  - xb flows through the Sync DMA queue, xt through the GpSimd queue, in
    superbatches of 4-8 tiles so the ~0.7 us per-DMA completion latency
    amortizes while the first compute starts early.
  - each sample is PADDED from 3136 to 3200 pixels (25 tiles of 128) with
    x=0. Pad pixels contribute 0 to a^T x and exactly 1/32 per cluster to
    the softmax weights, so a_sum overcounts by the constant 64/32 = 2.0,
    subtracted for free via the bias of the a_sum PSUM->SBUF copy. Every
    4-tile group then lies inside one sample: no row-split matmuls.
  - tiles are processed in GROUPS of 4: the four s tiles accumulate into
    one PSUM bank [128, 4, 32] and the softmax runs once per group
    (1 ACT exp + 1 DVE reduce + 1 DVE recip + 1 broadcast mul),
    amortizing per-instruction overheads 4x. a_sum is one matmul per
    group (lhsT = a4 [128, 4*32], rhs = ones) onto 128 PSUM partitions;
    a tiny fold matmul (eye(32) tiled 4x) in the epilogue sums the 4
    tile-slots.
  - mm2 emission lags mm1 by ~2 groups and is interleaved one unit per
    mm1-tile so the 215 ns acc-matmul streams hide mm1's LDWEIGHTS and
    the PE never waits on the softmax chain. Softmax skips
    max-subtraction (|s| <= ~10 is f32-exp-safe).
  - epilogue per sample: vT = C^T*a_sum + acc fused in one
    scalar_tensor_tensor, PE-transpose to [d, k], ACT square + DVE reduce
    for the intra-norm; the global L2 norm of the intra-normalized matrix
    is exactly sqrt(512), folded analytically into the Sqrt scale.
Measured end-to-end relative error vs the f32 reference ~1.0e-2
(fp8 on the a^T x path; the all-bf16 variant measures 1.8e-3).
"""
import sys

if '/opt/trn_rl_repo' not in sys.path:
    sys.path.insert(0, '/opt/trn_rl_repo')

from contextlib import ExitStack

import numpy as np

N_PIX = 3136
N_SAMP = 2
P = 128
NTS = 25              # tiles per sample (padded to 3200 px)
NT = NTS * N_SAMP     # 50
N_ROWSP = NT * P      # 6400 padded rows
D = 512
K = 32
DC = D // P           # 4
N_CORES = 8
MAXG = 4
MAXB = 4
KEEP = 10             # pending mm2 closures kept in flight (~2 groups)

# groups (sample, first local tile, n tiles); the last group of each
# sample is the single padded tile 24
GROUPS = []
for _s in range(N_SAMP):
    GROUPS += [(_s, 4 * i, 4) for i in range(6)] + [(_s, 24, 1)]

_cache = {}


def _build():
    import concourse.bacc as bacc
    import concourse.mybir as mybir
    import concourse.tile as tile
    from concourse.bass import ts

    F32 = mybir.dt.float32
    BF16 = mybir.dt.bfloat16
    FP8 = mybir.dt.float8e4
    MULT = mybir.AluOpType.mult
    ADD = mybir.AluOpType.add

    nc = bacc.Bacc("TRN2", target_bir_lowering=False, debug=False)

    xb = nc.declare_dram_parameter("xb", [N_ROWSP, D], BF16, isOutput=False)
    xt = nc.declare_dram_parameter("xt", [NT, P, DC, P], BF16,
                                   isOutput=False)
    wc = nc.declare_dram_parameter("wc", [D, K], BF16, isOutput=False)
    ct = nc.declare_dram_parameter("ct", [K, D], F32, isOutput=False)
    id32 = nc.declare_dram_parameter("id32", [K, K], F32, isOutput=False)
    ones2 = nc.declare_dram_parameter("ones2", [P, 2], BF16, isOutput=False)
    wfold = nc.declare_dram_parameter("wfold", [P, K], F32, isOutput=False)
    out = nc.declare_dram_parameter("out", [N_SAMP, DC, P, K], F32,
                                    isOutput=True)
    xb, xt, wc, ct, id32, ones2, wfold, out = (
        xb.ap(), xt.ap(), wc.ap(), ct.ap(), id32.ap(), ones2.ap(),
        wfold.ap(), out.ap())
    xb_r = xb.rearrange("(t p) d -> p t d", p=P)      # [P, NT, D]
    xt_r = xt.rearrange("t p j q -> p t j q")         # [P, NT, DC, P]

    with tile.TileContext(nc) as tc, ExitStack() as ctx:
        consts = ctx.enter_context(tc.tile_pool(name="consts", bufs=1))
        xbpool = ctx.enter_context(tc.tile_pool(name="xbpool", bufs=6))
        xtpool = ctx.enter_context(tc.tile_pool(name="xtpool", bufs=6))
        small = ctx.enter_context(tc.tile_pool(name="small", bufs=6))
        epil = ctx.enter_context(tc.tile_pool(name="epil", bufs=2))
        ps_s = ctx.enter_context(tc.tile_pool(name="ps_s", bufs=3,
                                              space="PSUM"))
        ps_acc = ctx.enter_context(tc.tile_pool(name="ps_acc", bufs=2,
                                                space="PSUM"))
        ps_asum = ctx.enter_context(tc.tile_pool(name="ps_asum", bufs=2,
                                                 space="PSUM"))

        wc_sb = consts.tile([P, DC, K], BF16)
        nc.sync.dma_start(out=wc_sb, in_=wc.rearrange("(c p) k -> p c k", p=P))
        ct_sb = consts.tile([K, D], F32)
        nc.sync.dma_start(out=ct_sb, in_=ct)
        id32_sb = consts.tile([K, K], F32)
        nc.sync.dma_start(out=id32_sb, in_=id32)
        ones_sb = consts.tile([P, 2], BF16)
        nc.sync.dma_start(out=ones_sb, in_=ones2)
        wfold_sb = consts.tile([P, K], F32)
        nc.sync.dma_start(out=wfold_sb, in_=wfold)

        # pre-load the ACT function tables (1.5 us each) in the startup
        # shadow so first-use loads don't land mid-kernel or in the tail
        for wf in (mybir.ActivationFunctionType.Exp,
                   mybir.ActivationFunctionType.Square,
                   mybir.ActivationFunctionType.Sqrt):
            warm = small.tile([P, 1], F32, name="warm")
            nc.scalar.activation(warm, ones_sb[:, 0:1], wf)

        acc = [ps_acc.tile([K, D], F32, name=f"acc{s}", tag="acc")
               for s in range(N_SAMP)]
        asum_ps = [ps_asum.tile([P, 2], F32, name=f"asumps{s}", tag="asum_ps")
                   for s in range(N_SAMP)]

        def epilogue(s):
            # fold the 4 tile-slot blocks of a_sum and subtract the pad
            # contribution (64 pad px * 1/32 = 2.0 total, 0.5 per block)
            asum_sb = epil.tile([P, 2], F32, name=f"asb{s}", tag="asb")
            nc.vector.tensor_scalar_add(asum_sb, asum_ps[s], -0.5)
            asum_f = ps_s.tile([K, 2], F32, name=f"af{s}", tag="sps")
            nc.tensor.matmul(asum_f, wfold_sb, asum_sb,
                             start=True, stop=True, skip_group_check=True)
            # vT = C^T * a_sum + acc, fused on DVE
            vt_sb = epil.tile([K, D], F32, name=f"vt{s}", tag="vt")
            nc.vector.scalar_tensor_tensor(vt_sb, ct_sb, asum_f[:, 0:1],
                                           acc[s][:, :], op0=MULT, op1=ADD)
            v_ps = ps_s.tile([P, DC, K], F32, name=f"vps{s}", tag="sps")
            for j in range(DC):
                nc.tensor.transpose(v_ps[:, j, :], vt_sb[:, ts(j, P)], id32_sb)
            vsq = epil.tile([P, DC, K], F32, name=f"vsq{s}", tag="vsq")
            nc.scalar.activation(vsq, v_ps,
                                 mybir.ActivationFunctionType.Square)
            ssq = epil.tile([P, DC], F32, name=f"ssq{s}", tag="ssq")
            nc.vector.reduce_sum(ssq, vsq, axis=mybir.AxisListType.X)
            snorm = epil.tile([P, DC], F32, name=f"sn{s}", tag="sn")
            nc.scalar.activation(snorm, ssq,
                                 mybir.ActivationFunctionType.Sqrt,
                                 scale=float(D))
            rmult = epil.tile([P, DC], F32, name=f"rm{s}", tag="rm")
            nc.vector.reciprocal(rmult, snorm)
            v_sb = epil.tile([P, DC, K], F32, name=f"v{s}", tag="v")
            nc.vector.tensor_mul(v_sb, v_ps, rmult.to_broadcast([P, DC, K]))
            nc.sync.dma_start(out=out[s].rearrange("c p k -> p c k"),
                              in_=v_sb)

        xt_t0_holder = [consts.tile([P, DC, P], BF16, name=f"xt0_{u}")
                        for u in range(MAXG)]

        pending = []

        def pop_pending(keep):
            while len(pending) > keep:
                pending.pop(0)()

        def make_acc(s, tl, a4, u, xb_t, bu):
            def emit():
                nc.tensor.matmul(acc[s][:, :], a4[:, u, :], xb_t[:, bu, :],
                                 start=(tl == 0), stop=(tl == NTS - 1),
                                 skip_group_check=True)
            return emit

        def make_asum(s, tl0, sz, a4):
            def emit():
                nc.tensor.matmul(asum_ps[s][0:sz * K, :], a4[:, 0:sz, :],
                                 ones_sb,
                                 start=(tl0 == 0), stop=(tl0 + sz == NTS),
                                 skip_group_check=True)
                if tl0 + sz == NTS:
                    epilogue(s)
            return emit

        for gi, (s, tl0, sz) in enumerate(GROUPS):
            t0 = s * NTS + tl0
            xb_t = xbpool.tile([P, MAXB, D], BF16, name="xb_t")
            xbq = nc.sync if gi % 2 == 0 else nc.scalar
            if gi == 0:
                # per-tile DMAs so the first mm1 starts after ~128 KB
                for u in range(sz):
                    nc.gpsimd.dma_start(out=xt_t0_holder[u][:, :, :],
                                        in_=xt_r[:, t0 + u, :, :])
                    xbq.dma_start(out=xb_t[:, u:u + 1, :],
                                  in_=xb_r[:, t0 + u:t0 + u + 1, :])
            else:
                xbq.dma_start(out=xb_t[:, 0:sz, :],
                              in_=xb_r[:, t0:t0 + sz, :])
            if gi > 0:
                xt_t = xtpool.tile([P, MAXB, DC, P], BF16, name="xt_t")
                nc.gpsimd.dma_start(out=xt_t[:, 0:sz, :, :],
                                    in_=xt_r[:, t0:t0 + sz, :, :])
            s_ps = ps_s.tile([P, MAXG, K], F32, name="s_ps", tag="sps")
            for u in range(sz):
                xt_src = xt_t0_holder[u] if gi == 0 else xt_t[:, u, :, :]
                for j in range(DC):
                    nc.tensor.matmul(s_ps[:, u, :], xt_src[:, j, :],
                                     wc_sb[:, j, :],
                                     start=(j == 0), stop=(j == DC - 1),
                                     skip_group_check=True)
                pop_pending(KEEP)
            exp4 = small.tile([P, MAXG, K], F32, name="exp4")
            nc.scalar.activation(exp4[:, 0:sz, :], s_ps[:, 0:sz, :],
                                 mybir.ActivationFunctionType.Exp)
            sum4 = small.tile([P, MAXG], F32, name="sum4")
            nc.vector.reduce_sum(sum4[:, 0:sz], exp4[:, 0:sz, :],
                                 axis=mybir.AxisListType.X)
            rcp4 = small.tile([P, MAXG], F32, name="rcp4")
            nc.vector.reciprocal(rcp4[:, 0:sz], sum4[:, 0:sz])
            a4 = small.tile([P, MAXG, K], BF16, name="a4")
            nc.vector.tensor_mul(
                a4[:, 0:sz, :], exp4[:, 0:sz, :],
                rcp4[:, 0:sz].to_broadcast([P, sz, K]))
            for u in range(sz):
                pending.append(make_acc(s, tl0 + u, a4, u, xb_t, u))
            pending.append(make_asum(s, tl0, sz, a4))
        pop_pending(0)

    nc.finalize()
    return nc


def _get_nc():
    if "nc" not in _cache:
        _cache["nc"] = _build()
    return _cache["nc"]


def make_maps(x, Wc, C):
    """Host-side prep: shard over batch, pad samples to 3200 px, build
    fp8 xb / pre-transposed bf16 xt."""
    import ml_dtypes

    bf16 = ml_dtypes.bfloat16
    x = np.asarray(x, dtype=np.float32)
    wc_h = np.asarray(Wc, dtype=np.float32).astype(bf16)
    ct_h = np.ascontiguousarray(np.asarray(C, dtype=np.float32).T)
    id32 = np.eye(K, dtype=np.float32)
    ones2 = np.ones((P, 2), dtype=bf16)
    wfold_h = np.tile(np.eye(K, dtype=np.float32), (DC, 1))

    B = x.shape[0]
    per = B // N_CORES
    maps = []
    for i in range(N_CORES):
        xs = x[i * per:(i + 1) * per].reshape(per, N_PIX, D)
        xp = np.zeros((per, NTS * P, D), dtype=np.float32)
        xp[:, :N_PIX, :] = xs
        xp = xp.reshape(N_ROWSP, D)
        # xt[t, p, j, q] = xp[128t+q, 128j+p]
        xtt = np.ascontiguousarray(
            xp.reshape(NT, P, DC, P).transpose(0, 3, 2, 1).astype(bf16))
        maps.append({"xb": np.ascontiguousarray(xp.astype(bf16)), "xt": xtt,
                     "wc": wc_h, "ct": ct_h, "id32": id32, "ones2": ones2,
                     "wfold": wfold_h})
    return maps


def kernel(x, Wc, C):
    from concourse.bass_utils import run_bass_kernel_spmd

    nc = _get_nc()
    maps = make_maps(x, Wc, C)
    res = run_bass_kernel_spmd(nc, maps, list(range(N_CORES)))
    outs = [r["out"].reshape(N_SAMP, D * K) for r in res.results]
    return np.concatenate(outs, axis=0)
